# revision 1
# baseline (speedup 1.0000x reference)
"""Trainium2 Bass kernel for nn_CholeskyConstraintLayer.

Maps x:(B,16) f32 -> rho:(B,4,4,2) f32 where rho = L L^dagger / (trace + eps),
L lower-triangular complex 4x4 built from x (softplus diagonal, raw re/im
off-diagonals).

Per-sample math (y = x with softplus applied at flat positions 0,3,8,15):
  row0 = y[0:1], row1 = y[1:4], row2 = y[4:9], row3 = y[9:16]   (interleaved re,im; diag last)
  rho_ij_re (i>=j) = dot(row_i[0:2j+1], row_j[0:2j+1])
  rho_ij_im (i>j)  = dot(zrow_i[0:2j+1], row_j[0:2j+1])  with z = pair-swapped,
                     odd-negated y (z[2k]=y[2k+1], z[2k+1]=-y[2k] inside each row)
  diag: rho_ii = sum of squares of row_i;  trace = sum of all 16 squares.

Work split per tile (samples on partitions x free dim, 16 values contiguous per
sample): ACT does softplus/squares/z-build/upper-triangle fills; DVE does the
dot-product multiplies+adds+segment reduces+reciprocal; POOL (gpsimd) does the
j=0 column products, the in-place normalize and the zero diagonal-imag slots.
DMA via HWDGE (nc.sync) so it never contends with compute.

Data parallel over 8 NeuronCores: batch padded to 8*128*sum(F_LIST) samples,
each core gets one contiguous shard.
"""

import numpy as np

P = 128
EPS = 1e-8
N_CORES = 8
BATCH = 1_000_000
# Tapered per-tile free sizes (samples per partition per tile). Small head
# tiles fill the pipeline quickly; sum * P * N_CORES >= BATCH.
F_LIST = [60, 130, 150, 150, 150, 150, 97, 90]  # sum = 977
S_CORE = P * sum(F_LIST)  # 125056
S_PAD = S_CORE * N_CORES  # 1000448

_NC_CACHE = {}


def _emit(tc, x_ap, out_ap, f_list):
    import concourse.bass as bass
    import concourse.mybir as mybir
    from contextlib import ExitStack

    nc = tc.nc
    f32 = mybir.dt.float32
    A = mybir.AluOpType
    ACT = mybir.ActivationFunctionType

    def block_ap(view3, start, bstride, nblocks, inner, bcast=False):
        """(128,F,inner) slice at col `start` -> (128,F,nblocks,inner) blocks."""
        a = view3[:, :, start:start + inner]
        dims = [list(d) for d in a.ap]
        step = 0 if bcast else bstride
        new = dims[:2] + [[step, nblocks]] + [dims[2]]
        return bass.AP(tensor=a.tensor, offset=a.offset, ap=new)

    with ExitStack() as ctx:
        tp = lambda name, bufs: ctx.enter_context(tc.tile_pool(name=name, bufs=bufs))
        ypool = tp("y", 3)
        sqpool = tp("sq", 3)
        zpool = tp("z", 3)
        prpool = tp("pr", 3)
        mpool = tp("misc", 3)
        opool = tp("out", 3)

        s0 = 0
        for ti, F in enumerate(f_list):
            # ---- DMA in: partition p holds samples s0 + p*F .. s0 + (p+1)*F-1
            y_t = ypool.tile([P, F * 16], f32, tag="y")
            xin = bass.AP(tensor=x_ap.tensor, offset=(s0 * 16),
                          ap=[[F * 16, P], [1, F * 16]])
            nc.sync.dma_start(y_t[:, :], xin)

            yf = y_t[:, :]
            y = yf.rearrange("p (f e) -> p f e", e=16)

            # ---- ACT: softplus = Ln(Exp(x) + 1) on diagonal positions, in
            # place (sq cols used as scratch for the exp; Square later
            # overwrites all of sq from the softplus'd y).
            sq_t = sqpool.tile([P, F * 16], f32, tag="sq")
            sq = sq_t[:, :].rearrange("p (f e) -> p f e", e=16)
            # diagonal cols pair into affine APs: (3,8) step 5, (0,15) step 15
            for sl in (slice(3, 9, 5), slice(0, 16, 15)):
                nc.scalar.activation(sq[:, :, sl], y[:, :, sl], ACT.Exp)
                nc.scalar.activation(y[:, :, sl], sq[:, :, sl], ACT.Ln, bias=1.0)

            # ---- ACT: squares of everything
            nc.scalar.activation(sq_t[:, :], yf, ACT.Square)

            # ---- ACT: z = swapped/negated pairs (cols: i20,-r20,i21, i30,-r30,i31,-r31,i32)
            z_t = zpool.tile([P, F * 8], f32, tag="z")
            z = z_t[:, :].rearrange("p (f e) -> p f e", e=8)
            nc.scalar.copy(z[:, :, 0:3:2], y[:, :, 5:8:2])     # z0=y5, z2=y7
            nc.scalar.copy(z[:, :, 3:8:2], y[:, :, 10:15:2])   # z3=y10, z5=y12, z7=y14
            nc.scalar.mul(z[:, :, 1:2], y[:, :, 4:5], -1.0)    # z1=-y4
            nc.scalar.mul(z[:, :, 4:7:2], y[:, :, 9:12:2], -1.0)  # z4=-y9, z6=-y11

            # ---- DVE: off-diagonal products (TT ISA allows max 3 AP dims,
            # so one op per 3-element block)
            pr_t = prpool.tile([P, F * 22], f32, tag="pr")
            prf = pr_t[:, :]
            prv = prf.rearrange("p (f e) -> p f e", e=22)
            row1 = y[:, :, 1:4]
            nc.vector.tensor_tensor(prv[:, :, 0:3], y[:, :, 4:7], row1, op=A.mult)
            nc.vector.tensor_tensor(prv[:, :, 3:6], y[:, :, 9:12], row1, op=A.mult)
            nc.vector.tensor_tensor(prv[:, :, 6:9], z[:, :, 0:3], row1, op=A.mult)
            nc.vector.tensor_tensor(prv[:, :, 9:12], z[:, :, 3:6], row1, op=A.mult)
            nc.vector.tensor_tensor(prv[:, :, 12:17], y[:, :, 9:14], y[:, :, 4:9], op=A.mult)
            nc.vector.tensor_tensor(prv[:, :, 17:22], z[:, :, 3:8], y[:, :, 4:9], op=A.mult)

            out_t = opool.tile([P, F * 32], f32, tag="out")
            ov = out_t[:, :].rearrange("p (f e) -> p f e", e=32)

            # ---- POOL: zero the diagonal-imag slots first (no data deps --
            # absorbs the out-slot release wait while other engines work)
            nc.gpsimd.memset(ov[:, :, 1:32:10], 0.0)

            # ---- POOL: j=0 products straight into (unnormalized) out slots
            y0b = y[:, :, 0:1].broadcast_to((P, F, 2))
            nc.gpsimd.tensor_tensor(ov[:, :, 8:10], y[:, :, 1:3], y0b, op=A.mult)
            nc.gpsimd.tensor_tensor(ov[:, :, 16:18], y[:, :, 4:6], y0b, op=A.mult)
            nc.gpsimd.tensor_tensor(ov[:, :, 24:26], y[:, :, 9:11], y0b, op=A.mult)

            # ---- DVE: adds. k3: (21re,31re,21im,31im) -> slots (18,26,19,27)
            misc_t = mpool.tile([P, F * 6], f32, tag="misc")
            mv = misc_t[:, :].rearrange("p (f e) -> p f e", e=6)
            pr4 = prv[:, :, 0:12].rearrange("p f (s e) -> p f s e", e=3)
            t3 = mv[:, :, 0:4]
            nc.vector.tensor_tensor(t3, pr4[:, :, :, 0], pr4[:, :, :, 1], op=A.add)
            # final adds: (21re,31re) -> slots (18,26); (21im,31im) -> (19,27)
            dst_re = bass.AP(tensor=ov.tensor, offset=ov.offset + 18,
                             ap=[list(ov.ap[0]), [32, F], [8, 2]])
            dst_im = bass.AP(tensor=ov.tensor, offset=ov.offset + 19,
                             ap=[list(ov.ap[0]), [32, F], [8, 2]])
            nc.vector.tensor_tensor(dst_re, t3[:, :, 0:2], pr4[:, :, 0:2, 2], op=A.add)
            nc.vector.tensor_tensor(dst_im, t3[:, :, 2:4], pr4[:, :, 2:4, 2], op=A.add)
            # k5: (32re,32im) -> slots (28,29)
            pr5 = prv[:, :, 12:22].rearrange("p f (s e) -> p f s e", e=5)
            t5 = mv[:, :, 4:6]
            nc.vector.tensor_tensor(t5, pr5[:, :, :, 0], pr5[:, :, :, 1], op=A.add)
            nc.vector.tensor_tensor(t5, t5, pr5[:, :, :, 2], op=A.add)
            nc.vector.tensor_tensor(t5, t5, pr5[:, :, :, 3], op=A.add)
            nc.vector.tensor_tensor(ov[:, :, 28:30], t5, pr5[:, :, :, 4], op=A.add)

            # ---- DVE: diagonal sums of squares -> slots 10, 20, 30
            X = mybir.AxisListType.X
            nc.vector.tensor_reduce(ov[:, :, 10:11], sq[:, :, 1:4], axis=X, op=A.add)
            nc.vector.tensor_reduce(ov[:, :, 20:21], sq[:, :, 4:9], axis=X, op=A.add)
            nc.vector.tensor_reduce(ov[:, :, 30:31], sq[:, :, 9:16], axis=X, op=A.add)

            # ---- DVE: trace + eps, reciprocal
            s1, s2, trE, rcp = (mv[:, :, 0:1], mv[:, :, 1:2], mv[:, :, 2:3], mv[:, :, 3:4])
            nc.vector.tensor_tensor(s1, sq[:, :, 0:1], ov[:, :, 10:11], op=A.add)
            nc.vector.tensor_tensor(s2, s1, ov[:, :, 20:21], op=A.add)
            nc.vector.scalar_tensor_tensor(trE, ov[:, :, 30:31], float(EPS), s2,
                                           op0=A.add, op1=A.add)
            nc.vector.reciprocal_approx_fast(rcp, trE)

            # ---- POOL: normalize lower triangle + diagonal (in place)
            nc.gpsimd.tensor_tensor(ov[:, :, 0:1], sq[:, :, 0:1], rcp, op=A.mult)
            nc.gpsimd.tensor_tensor(ov[:, :, 8:11], ov[:, :, 8:11],
                                    rcp.broadcast_to((P, F, 3)), op=A.mult)
            nc.gpsimd.tensor_tensor(ov[:, :, 16:21], ov[:, :, 16:21],
                                    rcp.broadcast_to((P, F, 5)), op=A.mult)
            nc.gpsimd.tensor_tensor(ov[:, :, 24:31], ov[:, :, 24:31],
                                    rcp.broadcast_to((P, F, 7)), op=A.mult)

            # ---- ACT: upper triangle from normalized lower (conjugate)
            nc.scalar.copy(ov[:, :, 2:7:2], ov[:, :, 8:25:8])      # re row0
            nc.scalar.copy(ov[:, :, 12:15:2], ov[:, :, 18:27:8])   # re (1,2),(1,3)
            nc.scalar.copy(ov[:, :, 22:23], ov[:, :, 28:29])       # re (2,3)
            nc.scalar.mul(ov[:, :, 3:8:2], ov[:, :, 9:26:8], -1.0)   # im row0
            nc.scalar.mul(ov[:, :, 13:16:2], ov[:, :, 19:28:8], -1.0)
            nc.scalar.mul(ov[:, :, 23:24], ov[:, :, 29:30], -1.0)

            # ---- DMA out
            odst = bass.AP(tensor=out_ap.tensor, offset=(s0 * 32),
                           ap=[[F * 32, P], [1, F * 32]])
            nc.sync.dma_start(odst, out_t[:, :])

            s0 += P * F


def _patch_act_tables():
    """Force every ACT function onto the one table set that contains all of
    Exp/Ln/Square/Copy, so the table-load pass emits a single load instead of
    ping-ponging exp_and_others <-> natural_log every tile (~2.7us per load).
    Keys keep their order so act_func_set_id indices stay valid."""
    import concourse.bacc as bacc
    from concourse.hw_specs import get_activation_tables as _orig

    if getattr(bacc, "_act_tables_patched", False):
        return

    def _patched(arch):
        t = _orig(arch)
        return {k: (v if k == "natural_log_exp_and_others" else set())
                for k, v in t.items()}

    bacc.get_activation_tables = _patched
    bacc._act_tables_patched = True


def _build_nc(f_list):
    import concourse.bacc as bacc
    import concourse.mybir as mybir
    import concourse.tile as tile

    _patch_act_tables()

    key = tuple(f_list)
    if key in _NC_CACHE:
        return _NC_CACHE[key]
    S = P * sum(f_list)
    nc = bacc.Bacc("TRN2", target_bir_lowering=False, debug=False)
    x = nc.dram_tensor("x", (S, 16), mybir.dt.float32, kind="ExternalInput")
    out = nc.dram_tensor("out", (S, 32), mybir.dt.float32, kind="ExternalOutput")
    with tile.TileContext(nc) as tc:
        _emit(tc, x.ap(), out.ap(), f_list)
    nc.compile()
    _NC_CACHE[key] = nc
    return nc


def kernel(x, _trace=False):
    from concourse.bass_utils import run_bass_kernel_spmd

    x = np.ascontiguousarray(np.asarray(x, dtype=np.float32))
    B = x.shape[0]
    assert x.shape == (B, 16) and B <= S_PAD
    xp = np.zeros((S_PAD, 16), dtype=np.float32)
    xp[:B] = x
    shards = xp.reshape(N_CORES, S_CORE, 16)
    nc = _build_nc(F_LIST)
    in_maps = [{"x": np.ascontiguousarray(shards[i])} for i in range(N_CORES)]
    res = run_bass_kernel_spmd(nc, in_maps, core_ids=list(range(N_CORES)),
                               trace=_trace)
    out = np.concatenate([r["out"].reshape(S_CORE, 32) for r in res.results], axis=0)
    result = out[:B].reshape(B, 4, 4, 2)
    if _trace:
        return result, res
    return result

